# revision 18
# baseline (speedup 1.0000x reference)
"""Trainium2 Bass kernel for nn_BaseLayer (MoE routing, 8 experts).

Strategy (expert-parallel, per the sharding hint):
  * Host computes the router exactly as the reference does (token-expert
    affinities + argmax + sigmoid gate) with jax-on-CPU so the assignment
    bit-matches the reference, then sorts tokens by expert.  In Bass all
    collectives must be compile-time static, so the dynamic
    dispatch/combine (all_to_all with runtime split sizes) is realized by
    the host sharding step: core e receives expert e's tokens, padded to a
    common capacity C so that a single NEFF runs SPMD on all 8 cores.
  * Each core runs the heavy part on device: LayerNorm -> FF1(+bias,relu)
    -> FF2 -> residual + sigmoid-gated combine, with the expert's weights
    resident in SBUF and all matmuls on the PE array.
  * Mixed-precision split-K: the first P1F k-tiles of FF1 and P2F k2-tiles
    of FF2 run as fp8e4(DoubleRow, 2 k-tiles/instr, ~1.8x rate); the rest
    stay bf16.  Activations and weights are pre-scaled (x16 / x64) so the
    fp8 operands sit in e4m3's normal range; the common 1/1024 descale is
    folded into the f32 activation scale / the host-side alpha.
  * ln_g / ln_b are folded into w1 / b1 on the host; b2 is applied on the
    host during unsharding (exact for the actual inputs where b2=0).

The output permutation is the inverse of the sort, so the final output is
independent of sort order; only the argmax assignment must match the
reference, which host-side jax-on-CPU guarantees.
"""

import numpy as np
import ml_dtypes

D = 1024   # embed dim
F = 4096   # ffn dim
E = 8      # experts == cores
P = 128    # partitions
KD = D // P        # 8  k-tiles over D
KF = F // P        # 32 k-tiles over F
GROUP_TILES = 2    # token tiles (of 128) processed per FF1 batch
NW = 4             # weight DMA chunks (consumption-ordered)
EPS = 1e-5

# fp8 split-K: first P1F of KD k-tiles (FF1) / P2F of KF k2-tiles (FF2)
# run in fp8e4 DoubleRow.  Error budget: rel_l2 ~ 3.2e-2 * sqrt(theta),
# theta = (P1F/KD + P2F/KF)/2; keep under the 2e-2 gate with margin.
P1F = 4            # fp8 k-tiles in FF1 (must be even)
P1B = KD - P1F     # bf16 k-tiles in FF1
P2F = 6            # fp8 k2-tiles in FF2 (must be even)
P2B = KF - P2F     # bf16 k2-tiles in FF2
SH = 16.0          # activation pre-scale (h)
SW = 64.0          # weight pre-scale
SA = 16.0          # FF1-output (aT) pre-scale


def _routing(x, centroids):
    """Affinity/argmax/alpha exactly like the reference (jax on CPU)."""
    try:
        import jax
        import jax.numpy as jnp

        cpu = jax.devices("cpu")[0]
        with jax.default_device(cpu):
            aff = jnp.asarray(x) @ jnp.asarray(centroids).T
            assign = jnp.argmax(aff, axis=1)
            alpha = jax.nn.sigmoid(
                jnp.take_along_axis(aff, assign[:, None], axis=1)
            )
            return np.asarray(assign), np.asarray(alpha)[:, 0].astype(np.float32)
    except Exception:
        aff = x.astype(np.float32) @ centroids.astype(np.float32).T
        assign = np.argmax(aff, axis=1)
        sel = np.take_along_axis(aff, assign[:, None], axis=1)[:, 0]
        alpha = 1.0 / (1.0 + np.exp(-sel.astype(np.float64)))
        return assign, alpha.astype(np.float32)


def _build(C):
    """Build the per-core Bass program for capacity C (multiple of 128)."""
    import concourse.bacc as bacc
    import concourse.bass as bass
    import concourse.mybir as mybir
    import concourse.tile as tile
    from concourse.masks import make_identity

    f32 = mybir.dt.float32
    bf16 = mybir.dt.bfloat16
    f8 = mybir.dt.float8e4
    AF = mybir.ActivationFunctionType
    ALU = mybir.AluOpType
    DR = mybir.MatmulPerfMode.DoubleRow

    nt = -(-C // P)                 # tiles, last may be partial
    last_rows = C - P * (nt - 1)
    def tile_rows(tt):
        return last_rows if tt == nt - 1 else P
    groups = []
    t = 0
    while t < nt:
        g = min(GROUP_TILES, nt - t)
        groups.append((t, g))
        t += g

    nc = bacc.Bacc("TRN2", target_bir_lowering=False, debug=False)
    xs_d = nc.dram_tensor("xs", [C, D], f32, kind="ExternalInput").ap()
    al_d = nc.dram_tensor("alphap", [P, nt], f32, kind="ExternalInput").ap()
    # fp8 weights come pre-interleaved from the host so that every
    # DoubleRow stationary/moving slice is a contiguous [2, .] block
    # (s3_lw dual-fp8 ldweights restriction).
    w1f_d = nc.dram_tensor("w1f", [P, P1F * F], f8, kind="ExternalInput").ap()
    w1b_d = nc.dram_tensor("w1b", [P1B * P, F], bf16, kind="ExternalInput").ap()
    w2f_d = nc.dram_tensor("w2f", [P, P2F * D], f8, kind="ExternalInput").ap()
    w2b_d = nc.dram_tensor("w2b", [P2B * P, D], bf16, kind="ExternalInput").ap()
    b1_d = nc.dram_tensor("b1p", [P, KF], f32, kind="ExternalInput").ap()
    out_d = nc.dram_tensor("out", [C, D], f32, kind="ExternalOutput").ap()

    with tile.TileContext(nc) as tc:
        with (
            tc.tile_pool(name="wpool", bufs=1) as wpool,
            tc.tile_pool(name="consts", bufs=1) as consts,
            tc.tile_pool(name="xsp", bufs=8) as xsp,
            tc.tile_pool(name="hp", bufs=1) as hp,
            tc.tile_pool(name="hTp", bufs=2) as hTp,
            tc.tile_pool(name="hT8p", bufs=2) as hT8p,
            tc.tile_pool(name="aTp", bufs=2) as aTp,
            tc.tile_pool(name="aT8p", bufs=2) as aT8p,
            tc.tile_pool(name="statp", bufs=3) as statp,
            tc.tile_pool(name="outp", bufs=2) as outp,
            tc.tile_pool(name="ptrp", bufs=2, space="PSUM") as ptrp,
            tc.tile_pool(name="pap", bufs=2, space="PSUM") as pap,
            tc.tile_pool(name="pyp", bufs=2, space="PSUM") as pyp,
        ):
            ident = consts.tile([P, P], bf16)
            make_identity(nc, ident)
            eps_t = consts.tile([P, 1], f32)
            nc.vector.memset(eps_t, EPS / (SH * SH))

            xs_tiles = {}

            def load_xs(tt, eng=None):
                # Late tiles reuse a pool slot, so their DMA carries a wait on
                # the slot's releasing combine.  Issue those from the idle
                # GpSimd queue: on the in-order Sync queue that wait would
                # head-block every later DMA issue (weights, outputs).
                eng = eng or nc.sync
                r = tile_rows(tt)
                xs_t = xsp.tile([P, D], f32, tag="xs", name=f"xs{tt}")
                eng.dma_start(xs_t[:r, 0:512], xs_d[tt * P:tt * P + r, 0:512])
                eng.dma_start(xs_t[:r, 512:1024], xs_d[tt * P:tt * P + r, 512:1024])
                xs_tiles[tt] = xs_t

            def emit_ln(tt, hT, hT8, ti):
                """Layernorm an already-loaded token tile, transpose into hT."""
                r = tile_rows(tt)
                xs_t = xs_tiles[tt]
                st = statp.tile([P, 2, 6], f32, tag="st")
                nc.vector.bn_stats(st[:r, 0, :], xs_t[:r, 0:512])
                nc.vector.bn_stats(st[:r, 1, :], xs_t[:r, 512:1024])
                mv = statp.tile([P, 2], f32, tag="mv")
                nc.vector.bn_aggr(mv[:r], st[:r])
                # mv[:,1] := SH/sqrt(var+eps)  (h pre-scaled by SH for fp8)
                nc.scalar.activation(
                    mv[:r, 1:2], mv[:r, 1:2], AF.Sqrt,
                    bias=eps_t[:r, 0:1], scale=1.0 / (SH * SH),
                )
                nc.vector.reciprocal(mv[:r, 1:2], mv[:r, 1:2])
                h_t = hp.tile([P, D], bf16, tag="h")
                # h = (x - mean) * rstd * SH  (pre-scaled by SH for fp8)
                nc.vector.tensor_scalar(
                    out=h_t[:r], in0=xs_t[:r],
                    scalar1=mv[:r, 0:1], scalar2=mv[:r, 1:2],
                    op0=ALU.subtract, op1=ALU.mult,
                )
                ptr = ptrp.tile([P, KD, P], bf16, tag="ptr")
                for k in range(KD):
                    nc.tensor.transpose(
                        ptr[:, k, :r], h_t[:r, k * P:(k + 1) * P],
                        ident[:r, :r],
                    )
                # first P1F k-tiles cast to fp8 for DoubleRow, rest bf16
                nc.vector.tensor_copy(
                    hT8[:, :, ti * P:ti * P + r], ptr[:, 0:P1F, :r]
                )
                nc.vector.tensor_copy(
                    hT[:, :, ti * P:ti * P + r], ptr[:, P1F:KD, :r]
                )

            def group_n(gidx):
                t0, gt = groups[gidx]
                return sum(tile_rows(t0 + ti) for ti in range(gt))

            def prep_group(gidx):
                t0, gt = groups[gidx]
                n = group_n(gidx)
                hT = hTp.tile([P, P1B, n], bf16, tag="hT")
                hT8 = hT8p.tile([P, P1F, n], f8, tag="hT8")
                for ti in range(gt):
                    emit_ln(t0 + ti, hT, hT8, ti)
                return hT, hT8

            # Front-load token DMA + layernorm + transpose for the first PRE
            # groups so their DMAs sit ahead of the bulk weight load in the
            # queues; PE starts FF1 as soon as w1 chunk 0 lands.
            PRE = min(2, len(groups))
            pre_tiles = min(PRE * GROUP_TILES, nt)
            for tt in range(pre_tiles):
                load_xs(tt)
            state = [prep_group(g) for g in range(PRE)]

            b1_t = consts.tile([P, KF], f32)
            nc.sync.dma_start(b1_t, b1_d)
            al_t = consts.tile([P, nt], f32)
            nc.sync.dma_start(al_t, al_d)

            # Expert weights, resident in SBUF, DMA'd in NW chunks ordered to
            # match first-group consumption order (chunked over F for w1).
            fw = F // NW
            mw = KF // NW           # m-tiles per w1 chunk
            # w1f: [p, m, k(2), j] so lhsT = w1fc[c][:, mm] is contiguous [2,128]
            w1fc = [wpool.tile([P, mw, P1F, P], f8, name=f"w1f{c}", tag=f"w1f{c}")
                    for c in range(NW)]
            w1bc = [wpool.tile([P, P1B, fw], bf16, name=f"w1b{c}", tag=f"w1b{c}")
                    for c in range(NW)]
            # w2f: [p, j, hh, k(2), col] so rhs = w2f[:, j, hh] is contiguous [2,512]
            w2f = wpool.tile([P, P2F // 2, 2, 2, 512], f8, name="w2f", tag="w2f")
            w2b = wpool.tile([P, P2B, D], bf16, name="w2b", tag="w2b")

            def load_w1(c):
                cs = mw * P1F * P
                nc.sync.dma_start(w1fc[c], w1f_d[:, c * cs:(c + 1) * cs])
                for k in range(P1B):
                    nc.sync.dma_start(
                        w1bc[c][:, k, :],
                        w1b_d[k * P:(k + 1) * P, c * fw:(c + 1) * fw],
                    )

            def load_w2f():
                js = 2 * 2 * 512    # elements per j-pair
                for j in range(P2F // 2):
                    nc.sync.dma_start(w2f[:, j], w2f_d[:, j * js:(j + 1) * js])

            w2b_splits = np.array_split(range(P2B), NW)

            def load_w2b(c):
                for k2 in w2b_splits[c]:
                    nc.sync.dma_start(w2b[:, k2, :], w2b_d[k2 * P:(k2 + 1) * P, :])

            # Queue order tracks first consumption: FF1(g0) eats w1 chunks
            # back to back, w2f is needed when FF2(g0) starts, the remaining
            # token tiles only when their group's layernorm runs.
            load_w1(0); load_w1(1); load_w1(2); load_w1(3)
            mid_tiles = min(pre_tiles + 4, nt)
            for tt in range(pre_tiles, mid_tiles):
                load_xs(tt)
            load_w2f()
            for tt in range(mid_tiles, nt):
                load_xs(tt, eng=nc.gpsimd)
            load_w2b(0); load_w2b(1); load_w2b(2); load_w2b(3)

            for gi, (t0, gt) in enumerate(groups):
                n = group_n(gi)
                hT, hT8 = state[gi]

                # FF1: aT[f, tok] = relu((h @ w1t).T / (SH*SW) + b1) * SA
                # fp8 k-pair via DoubleRow, then bf16 k-tiles.
                # fp8 k2-tiles of FF2 get aT as fp8 ([p,ti,j,k,tok] so the
                # DoubleRow lhsT slice is a contiguous [2,128]), rest bf16.
                aT8 = aT8p.tile([P, gt, P2F // 2, 2, P], f8, tag="aT8")
                aT = aTp.tile([P, P2B, n], bf16, tag="aT")
                for m in range(KF):
                    pa = pap.tile([P, n], f32, tag="pa")
                    cw, mm = divmod(m, KF // NW)
                    for q in range(P1F // 2):
                        nc.tensor.matmul(
                            pa,
                            lhsT=w1fc[cw][:, mm, 2 * q:2 * q + 2, :],
                            rhs=hT8[:, 2 * q:2 * q + 2, :],
                            start=(q == 0), stop=False, perf_mode=DR,
                        )
                    for k in range(P1B):
                        nc.tensor.matmul(
                            pa,
                            lhsT=w1bc[cw][:, k, mm * P:(mm + 1) * P],
                            rhs=hT[:, k, :],
                            start=False, stop=(k == P1B - 1),
                        )
                    if m < P2F:
                        j, kk = divmod(m, 2)
                        for ti in range(gt):
                            ri = tile_rows(t0 + ti)
                            nc.scalar.activation(
                                aT8[:, ti, j, kk, :ri],
                                pa[:, ti * P:ti * P + ri], AF.Relu,
                                bias=b1_t[:, m:m + 1], scale=SA / (SH * SW),
                            )
                    else:
                        nc.scalar.activation(
                            aT[:, m - P2F, :], pa, AF.Relu,
                            bias=b1_t[:, m:m + 1], scale=SA / (SH * SW),
                        )

                # FF2 + gated residual combine, per token tile
                for ti in range(gt):
                    tt = t0 + ti
                    r = tile_rows(tt)
                    off = ti * P
                    xs2_t = xs_tiles[tt]
                    py = pyp.tile([P, D], f32, tag="py")
                    for hh in range(2):
                        sl = slice(hh * 512, (hh + 1) * 512)
                        for j in range(P2F // 2):
                            nc.tensor.matmul(
                                py[:r, sl],
                                lhsT=aT8[:, ti, j, :, :r],
                                rhs=w2f[:, j, hh, :, :],
                                start=(j == 0), stop=False, perf_mode=DR,
                            )
                        for k2 in range(P2B):
                            nc.tensor.matmul(
                                py[:r, sl],
                                lhsT=aT[:, k2, off:off + r],
                                rhs=w2b[:, k2, sl],
                                start=False, stop=(k2 == P2B - 1),
                            )
                    # out = xs + (alpha/(SA*SW)) * py, in pipelined 512-halves
                    for hh in range(2):
                        sl = slice(hh * 512, (hh + 1) * 512)
                        o_h = outp.tile([P, 512], f32, tag="o")
                        nc.scalar.activation(
                            o_h[:r], py[:r, sl], AF.Copy, bias=0.0,
                            scale=al_t[:r, tt:tt + 1],
                        )
                        nc.vector.tensor_add(o_h[:r], o_h[:r], xs2_t[:r, sl])
                        # Early groups' outputs leave via the GpSimd issue
                        # path: on Sync their transfers queue behind the tail
                        # of the weight stream, and the stalled buffer
                        # recycling head-blocks the combine/relu chain.
                        for q in range(2):
                            # split issue paths: Sync and GpSimd queues drain
                            # the final output burst in parallel
                            oeng = nc.gpsimd if (gi < 2 or q == 1) else nc.sync
                            qs = slice(q * 256, (q + 1) * 256)
                            oeng.dma_start(
                                out_d[tt * P:tt * P + r, hh * 512 + q * 256:
                                      hh * 512 + (q + 1) * 256],
                                o_h[:r, qs],
                            )

                # Prepare group gi+PRE after the combines: its ACT sqrt then
                # sits behind this group's combines in ACT program order, so
                # a late token DMA cannot head-block the next group's relus;
                # its PE transposes run between FF2(gi) and FF1(gi+1).
                if gi + PRE < len(groups):
                    state.append(prep_group(gi + PRE))

    nc.compile()
    return nc


def _prepare(inputs):
    """Host routing + per-core input packing. Returns (in_maps, perm, meta)."""
    x = np.ascontiguousarray(
        np.asarray(inputs["input_features"], dtype=np.float32).reshape(-1, D)
    )
    cent = np.asarray(inputs["centroids"], np.float32)
    ln_g = np.asarray(inputs["ln_g"], np.float32)
    ln_b = np.asarray(inputs["ln_b"], np.float32)
    w1 = np.asarray(inputs["w1"], np.float32)
    b1 = np.asarray(inputs["b1"], np.float32)
    w2 = np.asarray(inputs["w2"], np.float32)

    assign, alpha = _routing(x, cent)
    counts = np.bincount(assign, minlength=E)
    order = np.argsort(assign, kind="stable")
    segs = np.concatenate([[0], np.cumsum(counts)])
    C = max(P, int(counts.max()))
    nt = -(-C // P)

    bf = ml_dtypes.bfloat16
    e4 = ml_dtypes.float8_e4m3
    in_maps = []
    perm = []
    for e in range(E):
        idx = order[segs[e]:segs[e + 1]]
        ne = len(idx)
        xs = np.zeros((C, D), np.float32)
        xs[:ne] = x[idx]
        al = np.zeros((nt * P,), np.float32)
        al[:ne] = alpha[idx] / (SA * SW)
        alphap = np.ascontiguousarray(al.reshape(nt, P).T)
        w1s = (w1[e] * ln_g[e][None, :]).T * SW          # [D, F], pre-scaled
        # interleave [k,p,m,j] -> [p, m, k, j] so each m-tile's dual-fp8
        # weight block [2,128] is contiguous in SBUF
        w1fe = np.ascontiguousarray(
            w1s[:P1F * P].astype(e4)
            .reshape(P1F, P, KF, P).transpose(1, 2, 0, 3).reshape(P, P1F * F)
        )
        w1be = np.ascontiguousarray(w1s[P1F * P:].astype(bf))
        w2s = w2[e].T * SW                               # [F, D], pre-scaled
        # interleave [j,k,p,hh,col] -> [p, j, hh, k, col] for contiguous
        # [2,512] moving blocks
        w2fe = np.ascontiguousarray(
            w2s[:P2F * P].astype(e4)
            .reshape(P2F // 2, 2, P, 2, 512).transpose(2, 0, 3, 1, 4)
            .reshape(P, P2F * D)
        )
        w2be = np.ascontiguousarray(w2s[P2F * P:].astype(bf))
        b1e = ((b1[e] + ln_b[e] @ w1[e].T) * SA).astype(np.float32)
        b1p = np.ascontiguousarray(b1e.reshape(KF, P).T)
        in_maps.append(
            {"xs": xs, "alphap": alphap, "w1f": w1fe, "w1b": w1be,
             "w2f": w2fe, "w2b": w2be, "b1p": b1p}
        )
        perm.append(idx)
    return in_maps, perm, (C, alpha)


def _unshard(inputs, results, perm, alpha):
    b2 = np.asarray(inputs["b2"], np.float32)
    x_shape = np.asarray(inputs["input_features"]).shape
    T = x_shape[0] * x_shape[1]
    out = np.empty((T, D), np.float32)
    for e in range(E):
        idx = perm[e]
        oe = np.asarray(results[e]["out"][:len(idx)], np.float32)
        if np.any(b2[e]):
            oe = oe + alpha[idx][:, None] * b2[e][None, :]
        out[idx] = oe
    return out.reshape(x_shape)


def run(inputs, **spmd_kwargs):
    """Full pipeline; returns (output, BassKernelResults, nc)."""
    from concourse.bass_utils import run_bass_kernel_spmd

    in_maps, perm, (C, alpha) = _prepare(inputs)
    nc = _build(C)
    res = run_bass_kernel_spmd(nc, in_maps, core_ids=list(range(E)), **spmd_kwargs)
    out = _unshard(inputs, res.results, perm, alpha)
    return out, res, nc


def kernel(**inputs) -> np.ndarray:
    out, _, _ = run(inputs)
    return out


# revision 21
# speedup vs baseline: 1.1860x; 1.1860x over previous
"""Trainium2 Bass kernel for nn_BaseLayer (MoE routing, 8 experts).

Strategy (expert-parallel, per the sharding hint):
  * Host computes the router exactly as the reference does (token-expert
    affinities + argmax + sigmoid gate) with jax-on-CPU so the assignment
    bit-matches the reference, then sorts tokens by expert.  In Bass all
    collectives must be compile-time static, so the dynamic
    dispatch/combine (all_to_all with runtime split sizes) is realized by
    the host sharding step: core e receives expert e's tokens, padded to a
    common capacity C so that a single NEFF runs SPMD on all 8 cores.
  * Each core runs the heavy part on device: LayerNorm -> FF1(+bias,relu)
    -> FF2 -> residual + sigmoid-gated combine, with the expert's weights
    resident in SBUF and all matmuls on the PE array.
  * Mixed-precision split-K: the first P1F k-tiles of FF1 and P2F k2-tiles
    of FF2 run as fp8e4(DoubleRow, 2 k-tiles/instr, ~1.8x rate); the rest
    stay bf16.  Activations and weights are pre-scaled (x16 / x64) so the
    fp8 operands sit in e4m3's normal range; the common 1/1024 descale is
    folded into the f32 activation scale / the host-side alpha.
  * ln_g / ln_b are folded into w1 / b1 on the host; b2 is applied on the
    host during unsharding (exact for the actual inputs where b2=0).

The output permutation is the inverse of the sort, so the final output is
independent of sort order; only the argmax assignment must match the
reference, which host-side jax-on-CPU guarantees.
"""

import numpy as np
import ml_dtypes

D = 1024   # embed dim
F = 4096   # ffn dim
E = 8      # experts == cores
P = 128    # partitions
KD = D // P        # 8  k-tiles over D
KF = F // P        # 32 k-tiles over F
GROUP_TILES = 2    # token tiles (of 128) processed per FF1 batch
NW = 4             # weight DMA chunks (consumption-ordered)
EPS = 1e-5

# fp8 split-K: first P1F of KD k-tiles (FF1) / P2F of KF k2-tiles (FF2)
# run in fp8e4 DoubleRow.  Error budget: rel_l2 ~ 3.2e-2 * sqrt(theta),
# theta = (P1F/KD + P2F/KF)/2; keep under the 2e-2 gate with margin.
P1F = 4            # fp8 k-tiles in FF1 (must be even)
P1B = KD - P1F     # bf16 k-tiles in FF1
P2F = 6            # fp8 k2-tiles in FF2 (must be even)
P2B = KF - P2F     # bf16 k2-tiles in FF2
SH = 16.0          # activation pre-scale (h)
SW = 64.0          # weight pre-scale
SA = 16.0          # FF1-output (aT) pre-scale


def _routing(x, centroids):
    """Affinity/argmax/alpha exactly like the reference (jax on CPU)."""
    try:
        import jax
        import jax.numpy as jnp

        cpu = jax.devices("cpu")[0]
        with jax.default_device(cpu):
            aff = jnp.asarray(x) @ jnp.asarray(centroids).T
            assign = jnp.argmax(aff, axis=1)
            alpha = jax.nn.sigmoid(
                jnp.take_along_axis(aff, assign[:, None], axis=1)
            )
            return np.asarray(assign), np.asarray(alpha)[:, 0].astype(np.float32)
    except Exception:
        aff = x.astype(np.float32) @ centroids.astype(np.float32).T
        assign = np.argmax(aff, axis=1)
        sel = np.take_along_axis(aff, assign[:, None], axis=1)[:, 0]
        alpha = 1.0 / (1.0 + np.exp(-sel.astype(np.float64)))
        return assign, alpha.astype(np.float32)


def _build(C):
    """Build the per-core Bass program for capacity C (multiple of 128)."""
    import concourse.bacc as bacc
    import concourse.bass as bass
    import concourse.mybir as mybir
    import concourse.tile as tile
    from concourse.masks import make_identity

    f32 = mybir.dt.float32
    bf16 = mybir.dt.bfloat16
    f8 = mybir.dt.float8e4
    AF = mybir.ActivationFunctionType
    ALU = mybir.AluOpType
    DR = mybir.MatmulPerfMode.DoubleRow

    nt = -(-C // P)                 # tiles, last may be partial
    last_rows = C - P * (nt - 1)
    def tile_rows(tt):
        return last_rows if tt == nt - 1 else P
    groups = []
    t = 0
    while t < nt:
        g = min(GROUP_TILES, nt - t)
        groups.append((t, g))
        t += g

    nc = bacc.Bacc("TRN2", target_bir_lowering=False, debug=False)
    xs_d = nc.dram_tensor("xs", [C, D], f32, kind="ExternalInput").ap()
    al_d = nc.dram_tensor("alphap", [P, nt], f32, kind="ExternalInput").ap()
    # fp8 weights come pre-interleaved from the host so that every
    # DoubleRow stationary/moving slice is a contiguous [2, .] block
    # (s3_lw dual-fp8 ldweights restriction).
    w1f_d = nc.dram_tensor("w1f", [P, P1F * F], f8, kind="ExternalInput").ap()
    w1b_d = nc.dram_tensor("w1b", [P1B * P, F], bf16, kind="ExternalInput").ap()
    w2f_d = nc.dram_tensor("w2f", [P, P2F * D], f8, kind="ExternalInput").ap()
    w2b_d = nc.dram_tensor("w2b", [P2B * P, D], bf16, kind="ExternalInput").ap()
    b1_d = nc.dram_tensor("b1p", [P, KF], f32, kind="ExternalInput").ap()
    out_d = nc.dram_tensor("out", [C, D], f32, kind="ExternalOutput").ap()

    with tile.TileContext(nc) as tc:
        with (
            tc.tile_pool(name="wpool", bufs=1) as wpool,
            tc.tile_pool(name="consts", bufs=1) as consts,
            tc.tile_pool(name="xsp", bufs=8) as xsp,
            tc.tile_pool(name="hp", bufs=1) as hp,
            tc.tile_pool(name="hTp", bufs=2) as hTp,
            tc.tile_pool(name="hT8p", bufs=2) as hT8p,
            tc.tile_pool(name="aTp", bufs=2) as aTp,
            tc.tile_pool(name="aT8p", bufs=2) as aT8p,
            tc.tile_pool(name="statp", bufs=3) as statp,
            tc.tile_pool(name="outp", bufs=2) as outp,
            tc.tile_pool(name="ptrp", bufs=2, space="PSUM") as ptrp,
            tc.tile_pool(name="pap", bufs=2, space="PSUM") as pap,
            tc.tile_pool(name="pyp", bufs=2, space="PSUM") as pyp,
        ):
            ident = consts.tile([P, P], bf16)
            make_identity(nc, ident)
            eps_t = consts.tile([P, 1], f32)
            nc.vector.memset(eps_t, EPS / (SH * SH))

            xs_tiles = {}

            def load_xs(tt, eng=None):
                # Late tiles reuse a pool slot, so their DMA carries a wait on
                # the slot's releasing combine.  Issue those from the idle
                # GpSimd queue: on the in-order Sync queue that wait would
                # head-block every later DMA issue (weights, outputs).
                eng = eng or nc.sync
                r = tile_rows(tt)
                xs_t = xsp.tile([P, D], f32, tag="xs", name=f"xs{tt}")
                eng.dma_start(xs_t[:r, 0:512], xs_d[tt * P:tt * P + r, 0:512])
                eng.dma_start(xs_t[:r, 512:1024], xs_d[tt * P:tt * P + r, 512:1024])
                xs_tiles[tt] = xs_t

            def emit_ln(tt, hT, hT8, ti):
                """Layernorm an already-loaded token tile, transpose into hT."""
                r = tile_rows(tt)
                xs_t = xs_tiles[tt]
                st = statp.tile([P, 2, 6], f32, tag="st")
                nc.vector.bn_stats(st[:r, 0, :], xs_t[:r, 0:512])
                nc.vector.bn_stats(st[:r, 1, :], xs_t[:r, 512:1024])
                mv = statp.tile([P, 2], f32, tag="mv")
                nc.vector.bn_aggr(mv[:r], st[:r])
                # mv[:,1] := SH/sqrt(var+eps)  (h pre-scaled by SH for fp8)
                nc.scalar.activation(
                    mv[:r, 1:2], mv[:r, 1:2], AF.Sqrt,
                    bias=eps_t[:r, 0:1], scale=1.0 / (SH * SH),
                )
                nc.vector.reciprocal(mv[:r, 1:2], mv[:r, 1:2])
                h_t = hp.tile([P, D], bf16, tag="h")
                # h = (x - mean) * rstd * SH  (pre-scaled by SH for fp8)
                nc.vector.tensor_scalar(
                    out=h_t[:r], in0=xs_t[:r],
                    scalar1=mv[:r, 0:1], scalar2=mv[:r, 1:2],
                    op0=ALU.subtract, op1=ALU.mult,
                )
                ptr = ptrp.tile([P, KD, P], bf16, tag="ptr")
                for k in range(KD):
                    nc.tensor.transpose(
                        ptr[:, k, :r], h_t[:r, k * P:(k + 1) * P],
                        ident[:r, :r],
                    )
                # first P1F k-tiles cast to fp8 for DoubleRow, rest bf16
                nc.vector.tensor_copy(
                    hT8[:, :, ti * P:ti * P + r], ptr[:, 0:P1F, :r]
                )
                nc.vector.tensor_copy(
                    hT[:, :, ti * P:ti * P + r], ptr[:, P1F:KD, :r]
                )

            def group_n(gidx):
                t0, gt = groups[gidx]
                return sum(tile_rows(t0 + ti) for ti in range(gt))

            def prep_group(gidx):
                t0, gt = groups[gidx]
                n = group_n(gidx)
                hT = hTp.tile([P, P1B, n], bf16, tag="hT")
                hT8 = hT8p.tile([P, P1F, n], f8, tag="hT8")
                for ti in range(gt):
                    emit_ln(t0 + ti, hT, hT8, ti)
                return hT, hT8

            # Front-load token DMA + layernorm + transpose for the first PRE
            # groups so their DMAs sit ahead of the bulk weight load in the
            # queues; PE starts FF1 as soon as w1 chunk 0 lands.
            PRE = min(2, len(groups))
            pre_tiles = min(PRE * GROUP_TILES, nt)
            for tt in range(pre_tiles):
                load_xs(tt)
            state = [prep_group(g) for g in range(PRE)]

            b1_t = consts.tile([P, KF], f32)
            nc.sync.dma_start(b1_t, b1_d)
            al_t = consts.tile([P, nt], f32)
            nc.sync.dma_start(al_t, al_d)

            # Expert weights, resident in SBUF, DMA'd in NW chunks ordered to
            # match first-group consumption order (chunked over F for w1).
            fw = F // NW
            mw = KF // NW           # m-tiles per w1 chunk
            # w1f: [p, m, k(2), j] so lhsT = w1fc[c][:, mm] is contiguous [2,128]
            w1fc = [wpool.tile([P, mw, P1F, P], f8, name=f"w1f{c}", tag=f"w1f{c}")
                    for c in range(NW)]
            w1bc = [wpool.tile([P, P1B, fw], bf16, name=f"w1b{c}", tag=f"w1b{c}")
                    for c in range(NW)]
            # w2f: [p, j, hh, k(2), col] so rhs = w2f[:, j, hh] is contiguous [2,512]
            w2f = wpool.tile([P, P2F // 2, 2, 2, 512], f8, name="w2f", tag="w2f")
            w2b_splits = np.array_split(np.arange(P2B), NW)
            w2bc = [wpool.tile([P, len(s), D], bf16, name=f"w2b{c}", tag=f"w2b{c}")
                    for c, s in enumerate(w2b_splits)]
            w2b_loc = {int(k2): (c, i) for c, s in enumerate(w2b_splits)
                       for i, k2 in enumerate(s)}

            def load_w1(c):
                cs = mw * P1F * P
                nc.sync.dma_start(w1fc[c], w1f_d[:, c * cs:(c + 1) * cs])
                for k in range(P1B):
                    nc.sync.dma_start(
                        w1bc[c][:, k, :],
                        w1b_d[k * P:(k + 1) * P, c * fw:(c + 1) * fw],
                    )

            def load_w2f():
                js = 2 * 2 * 512    # elements per j-pair
                for j in range(P2F // 2):
                    nc.sync.dma_start(w2f[:, j], w2f_d[:, j * js:(j + 1) * js])

            def load_w2b(c):
                for i, k2 in enumerate(w2b_splits[c]):
                    nc.sync.dma_start(w2bc[c][:, i, :], w2b_d[k2 * P:(k2 + 1) * P, :])

            # Queue order tracks first consumption: FF1(g0) eats w1 chunks
            # back to back, w2f is needed when FF2(g0) starts, the remaining
            # token tiles only when their group's layernorm runs.
            load_w1(0); load_w1(1); load_w1(2); load_w1(3)
            mid_tiles = min(pre_tiles + 4, nt)
            for tt in range(pre_tiles, mid_tiles):
                load_xs(tt)
            load_w2f()
            for tt in range(mid_tiles, nt):
                load_xs(tt, eng=nc.gpsimd)
            load_w2b(0); load_w2b(1); load_w2b(2); load_w2b(3)

            for gi, (t0, gt) in enumerate(groups):
                n = group_n(gi)
                hT, hT8 = state[gi]

                # FF1: aT[f, tok] = relu((h @ w1t).T / (SH*SW) + b1) * SA
                # fp8 k-pair via DoubleRow, then bf16 k-tiles.
                # fp8 k2-tiles of FF2 get aT as fp8 ([p,ti,j,k,tok] so the
                # DoubleRow lhsT slice is a contiguous [2,128]), rest bf16.
                aT8 = aT8p.tile([P, gt, P2F // 2, 2, P], f8, tag="aT8")
                aT = aTp.tile([P, P2B, n], bf16, tag="aT")
                for m in range(KF):
                    pa = pap.tile([P, n], f32, tag="pa")
                    cw, mm = divmod(m, KF // NW)
                    for q in range(P1F // 2):
                        nc.tensor.matmul(
                            pa,
                            lhsT=w1fc[cw][:, mm, 2 * q:2 * q + 2, :],
                            rhs=hT8[:, 2 * q:2 * q + 2, :],
                            start=(q == 0), stop=False, perf_mode=DR,
                        )
                    for k in range(P1B):
                        nc.tensor.matmul(
                            pa,
                            lhsT=w1bc[cw][:, k, mm * P:(mm + 1) * P],
                            rhs=hT[:, k, :],
                            start=False, stop=(k == P1B - 1),
                        )
                    if m < P2F:
                        j, kk = divmod(m, 2)
                        for ti in range(gt):
                            ri = tile_rows(t0 + ti)
                            nc.scalar.activation(
                                aT8[:, ti, j, kk, :ri],
                                pa[:, ti * P:ti * P + ri], AF.Relu,
                                bias=b1_t[:, m:m + 1], scale=SA / (SH * SW),
                            )
                    else:
                        nc.scalar.activation(
                            aT[:, m - P2F, :], pa, AF.Relu,
                            bias=b1_t[:, m:m + 1], scale=SA / (SH * SW),
                        )

                # FF2 + gated residual combine, per token tile
                for ti in range(gt):
                    tt = t0 + ti
                    r = tile_rows(tt)
                    off = ti * P
                    xs2_t = xs_tiles[tt]
                    py = pyp.tile([P, D], f32, tag="py")
                    for hh in range(2):
                        sl = slice(hh * 512, (hh + 1) * 512)
                        for j in range(P2F // 2):
                            nc.tensor.matmul(
                                py[:r, sl],
                                lhsT=aT8[:, ti, j, :, :r],
                                rhs=w2f[:, j, hh, :, :],
                                start=(j == 0), stop=False, perf_mode=DR,
                            )
                        for k2 in range(P2B):
                            cw, kk = w2b_loc[k2]
                            nc.tensor.matmul(
                                py[:r, sl],
                                lhsT=aT[:, k2, off:off + r],
                                rhs=w2bc[cw][:, kk, sl],
                                start=False, stop=(k2 == P2B - 1),
                            )
                    # out = xs + (alpha/(SA*SW)) * py, in pipelined 512-halves
                    for hh in range(2):
                        sl = slice(hh * 512, (hh + 1) * 512)
                        o_h = outp.tile([P, 512], f32, tag="o")
                        nc.scalar.activation(
                            o_h[:r], py[:r, sl], AF.Copy, bias=0.0,
                            scale=al_t[:r, tt:tt + 1],
                        )
                        nc.vector.tensor_add(o_h[:r], o_h[:r], xs2_t[:r, sl])
                        # Early groups' outputs leave via the GpSimd issue
                        # path: on Sync their transfers queue behind the tail
                        # of the weight stream, and the stalled buffer
                        # recycling head-blocks the combine/relu chain.
                        for q in range(2):
                            # split issue paths: Sync and GpSimd queues drain
                            # the final output burst in parallel
                            oeng = nc.gpsimd if (gi < 2 or q == 1) else nc.sync
                            qs = slice(q * 256, (q + 1) * 256)
                            oeng.dma_start(
                                out_d[tt * P:tt * P + r, hh * 512 + q * 256:
                                      hh * 512 + (q + 1) * 256],
                                o_h[:r, qs],
                            )

                # Prepare group gi+PRE after the combines: its ACT sqrt then
                # sits behind this group's combines in ACT program order, so
                # a late token DMA cannot head-block the next group's relus;
                # its PE transposes run between FF2(gi) and FF1(gi+1).
                if gi + PRE < len(groups):
                    state.append(prep_group(gi + PRE))

    nc.compile()
    return nc


def _prepare(inputs):
    """Host routing + per-core input packing. Returns (in_maps, perm, meta)."""
    x = np.ascontiguousarray(
        np.asarray(inputs["input_features"], dtype=np.float32).reshape(-1, D)
    )
    cent = np.asarray(inputs["centroids"], np.float32)
    ln_g = np.asarray(inputs["ln_g"], np.float32)
    ln_b = np.asarray(inputs["ln_b"], np.float32)
    w1 = np.asarray(inputs["w1"], np.float32)
    b1 = np.asarray(inputs["b1"], np.float32)
    w2 = np.asarray(inputs["w2"], np.float32)

    assign, alpha = _routing(x, cent)
    counts = np.bincount(assign, minlength=E)
    order = np.argsort(assign, kind="stable")
    segs = np.concatenate([[0], np.cumsum(counts)])
    C = max(P, int(counts.max()))
    nt = -(-C // P)

    bf = ml_dtypes.bfloat16
    e4 = ml_dtypes.float8_e4m3
    in_maps = []
    perm = []
    for e in range(E):
        idx = order[segs[e]:segs[e + 1]]
        ne = len(idx)
        xs = np.zeros((C, D), np.float32)
        xs[:ne] = x[idx]
        al = np.zeros((nt * P,), np.float32)
        al[:ne] = alpha[idx] / (SA * SW)
        alphap = np.ascontiguousarray(al.reshape(nt, P).T)
        w1s = (w1[e] * ln_g[e][None, :]).T * SW          # [D, F], pre-scaled
        # interleave [k,p,m,j] -> [p, m, k, j] so each m-tile's dual-fp8
        # weight block [2,128] is contiguous in SBUF
        w1fe = np.ascontiguousarray(
            w1s[:P1F * P].astype(e4)
            .reshape(P1F, P, KF, P).transpose(1, 2, 0, 3).reshape(P, P1F * F)
        )
        w1be = np.ascontiguousarray(w1s[P1F * P:].astype(bf))
        w2s = w2[e].T * SW                               # [F, D], pre-scaled
        # interleave [j,k,p,hh,col] -> [p, j, hh, k, col] for contiguous
        # [2,512] moving blocks
        w2fe = np.ascontiguousarray(
            w2s[:P2F * P].astype(e4)
            .reshape(P2F // 2, 2, P, 2, 512).transpose(2, 0, 3, 1, 4)
            .reshape(P, P2F * D)
        )
        w2be = np.ascontiguousarray(w2s[P2F * P:].astype(bf))
        b1e = ((b1[e] + ln_b[e] @ w1[e].T) * SA).astype(np.float32)
        b1p = np.ascontiguousarray(b1e.reshape(KF, P).T)
        in_maps.append(
            {"xs": xs, "alphap": alphap, "w1f": w1fe, "w1b": w1be,
             "w2f": w2fe, "w2b": w2be, "b1p": b1p}
        )
        perm.append(idx)
    return in_maps, perm, (C, alpha)


def _unshard(inputs, results, perm, alpha):
    b2 = np.asarray(inputs["b2"], np.float32)
    x_shape = np.asarray(inputs["input_features"]).shape
    T = x_shape[0] * x_shape[1]
    out = np.empty((T, D), np.float32)
    for e in range(E):
        idx = perm[e]
        oe = np.asarray(results[e]["out"][:len(idx)], np.float32)
        if np.any(b2[e]):
            oe = oe + alpha[idx][:, None] * b2[e][None, :]
        out[idx] = oe
    return out.reshape(x_shape)


def run(inputs, **spmd_kwargs):
    """Full pipeline; returns (output, BassKernelResults, nc)."""
    from concourse.bass_utils import run_bass_kernel_spmd

    in_maps, perm, (C, alpha) = _prepare(inputs)
    nc = _build(C)
    res = run_bass_kernel_spmd(nc, in_maps, core_ids=list(range(E)), **spmd_kwargs)
    out = _unshard(inputs, res.results, perm, alpha)
    return out, res, nc


def kernel(**inputs) -> np.ndarray:
    out, _, _ = run(inputs)
    return out
